# revision 2
# baseline (speedup 1.0000x reference)
"""Trainium2 Bass kernel for nn_CrossAttention (B=4, SQ=1024, SKV=2048, D=1024, H=16).

Sharding: core c handles batch b = c//2 and head-group g = c%2 (8 heads each).
Per core (all matmuls in bf16 with f32 PSUM accumulation):
  qT  = qw_g^T @ x_b^T          [512d, 1024m]   (d-major so heads slice on partitions)
  kT  = kw_g^T @ enc_b^T        [512d, 2048n]
  V   = enc_b @ vw_g (+vb)      [2048n, 512d]   (+ a column of ones per head)
  per head h (d-slice of 64):
    S^T[n,m] = kT_h^T.T @ qT_h  (K=64 matmuls, scores transposed: partitions=skv)
    E^T      = exp(0.125*S^T + mask[n])          (ACT, bias = per-partition mask)
    [O^T | Z] = [V_h | 1].T @ E^T                (PV matmul; row 64 = softmax denom Z[m])
    P^T      = E^T * (1/Z[m])   -> attn_weights output (stored transposed; host views back)
    attnT_h  = O^T * (1/Z[m])   (normalization folded after PV)
  out_partial = attnT.T @ pw_g  [1024m, 1024]   (host sums the 2 partials per batch + bias)

Outputs per core: ptout [8, 2048, 1024] f32 (P^T per head), outp [1024, 1024] f32.
Host: attn_weights[b, g*8+h] = ptout[h].T (numpy view), attn_output = sum of partials + bias.
"""
import sys

if "/opt/trn_rl_repo" not in sys.path:
    sys.path.insert(0, "/opt/trn_rl_repo")

import numpy as np
import ml_dtypes

import concourse.bass as bass
import concourse.tile as tile
from concourse import bacc, mybir
from concourse.bass_utils import run_bass_kernel_spmd

BF16 = mybir.dt.bfloat16
F32 = mybir.dt.float32
AF = mybir.ActivationFunctionType

B, SQ, SKV, D, H = 4, 1024, 2048, 1024, 16
DL = D // 2          # 512 local d per core (8 heads)
KC = D // 128        # 8 contraction chunks
NCH = SKV // 128     # 16 skv chunks
MCH = SQ // 128      # 8 sq chunks
DCH = DL // 128      # 4 local-d chunks
HL = 8               # heads per core

_NC = None


def _build():
    nc = bacc.Bacc(None, target_bir_lowering=False)

    xT = nc.dram_tensor("xT", [D, SQ], BF16, kind="ExternalInput")
    encT = nc.dram_tensor("encT", [D, SKV], BF16, kind="ExternalInput")
    qw = nc.dram_tensor("qw", [D, DL], BF16, kind="ExternalInput")
    kw = nc.dram_tensor("kw", [D, DL], BF16, kind="ExternalInput")
    vw = nc.dram_tensor("vw", [D, DL], BF16, kind="ExternalInput")
    pw = nc.dram_tensor("pw", [DL, D], BF16, kind="ExternalInput")
    qb = nc.dram_tensor("qb", [128, DCH], F32, kind="ExternalInput")
    kb = nc.dram_tensor("kb", [128, DCH], F32, kind="ExternalInput")
    vb = nc.dram_tensor("vb", [1, DL], F32, kind="ExternalInput")
    mask = nc.dram_tensor("mask", [128, NCH], F32, kind="ExternalInput")

    ptout = nc.dram_tensor("ptout", [HL, SKV, SQ], F32, kind="ExternalOutput")
    outp = nc.dram_tensor("outp", [SQ, D], F32, kind="ExternalOutput")

    with tile.TileContext(nc) as tc:
        import contextlib
        with contextlib.ExitStack() as ctx:
            persist = ctx.enter_context(tc.tile_pool(name="persist", bufs=1))
            inpool = ctx.enter_context(tc.tile_pool(name="inputs", bufs=1))
            work = ctx.enter_context(tc.tile_pool(name="work", bufs=2))
            etp = ctx.enter_context(tc.tile_pool(name="et", bufs=18))
            outs = ctx.enter_context(tc.tile_pool(name="outs", bufs=3))
            psum = ctx.enter_context(tc.tile_pool(name="psum", bufs=2, space="PSUM"))

            # ---- constants / small inputs ----
            ones_f = persist.tile([1, 128], F32, tag="ones_f")
            nc.vector.memset(ones_f, 1.0)
            qb_sb = persist.tile([128, DCH], F32, tag="qb")
            kb_sb = persist.tile([128, DCH], F32, tag="kb")
            vb_sb = persist.tile([1, DL], F32, tag="vb")
            mask_sb = persist.tile([128, NCH], F32, tag="mask")
            nc.sync.dma_start(out=qb_sb, in_=qb[:, :])
            nc.sync.dma_start(out=kb_sb, in_=kb[:, :])
            nc.sync.dma_start(out=vb_sb, in_=vb[:, :])
            nc.sync.dma_start(out=mask_sb, in_=mask[:, :])

            # ---- big inputs ----
            xT_sb = inpool.tile([128, KC, SQ], BF16, tag="xT")
            encT_sb = inpool.tile([128, KC, SKV], BF16, tag="encT")
            qw_sb = inpool.tile([128, KC, DL], BF16, tag="qw")
            kw_sb = inpool.tile([128, KC, DL], BF16, tag="kw")
            vw_sb = inpool.tile([128, KC, DL], BF16, tag="vw")
            pw_sb = persist.tile([128, DCH, D], BF16, tag="pw")
            nc.sync.dma_start(out=kw_sb, in_=kw[:, :].rearrange("(kc p) d -> p kc d", p=128))
            nc.sync.dma_start(out=qw_sb, in_=qw[:, :].rearrange("(kc p) d -> p kc d", p=128))
            nc.sync.dma_start(out=encT_sb, in_=encT[:, :].rearrange("(kc p) n -> p kc n", p=128))
            nc.sync.dma_start(out=xT_sb, in_=xT[:, :].rearrange("(kc p) m -> p kc m", p=128))
            nc.sync.dma_start(out=vw_sb, in_=vw[:, :].rearrange("(kc p) d -> p kc d", p=128))
            nc.sync.dma_start(out=pw_sb, in_=pw[:, :].rearrange("(dc p) n -> p dc n", p=128))

            # ---- persistent intermediates ----
            qT_sb = persist.tile([128, DCH, SQ], BF16, tag="qT")
            kT_sb = persist.tile([128, DCH, SKV], BF16, tag="kT")
            V_sb = persist.tile([128, NCH, HL, 65], BF16, tag="V")
            attnT_sb = persist.tile([128, DCH, SQ], BF16, tag="attnT")
            nc.vector.memset(V_sb, 1.0)  # ones column (idx 64) survives the V copies

            def proj_kT_qT(dc):
                # kT[:, dc, :]: four 512-wide column groups, 8 K-chunks each
                for nh in range(SKV // 512):
                    ps = psum.tile([128, 1024], F32, tag="big")
                    for kc in range(KC):
                        nc.tensor.matmul(
                            ps[:, 0:512],
                            lhsT=kw_sb[:, kc, dc * 128:(dc + 1) * 128],
                            rhs=encT_sb[:, kc, nh * 512:(nh + 1) * 512],
                            start=(kc == 0), stop=(kc == KC - 1))
                    nc.scalar.activation(
                        out=kT_sb[:, dc, nh * 512:(nh + 1) * 512], in_=ps[:, 0:512],
                        func=AF.Identity, bias=kb_sb[:, dc:dc + 1], scale=1.0)
                ps = psum.tile([128, 1024], F32, tag="big")
                for mh in range(2):
                    for kc in range(KC):
                        nc.tensor.matmul(
                            ps[:, mh * 512:(mh + 1) * 512],
                            lhsT=qw_sb[:, kc, dc * 128:(dc + 1) * 128],
                            rhs=xT_sb[:, kc, mh * 512:(mh + 1) * 512],
                            start=(kc == 0), stop=(kc == KC - 1))
                nc.scalar.activation(
                    out=qT_sb[:, dc, :], in_=ps,
                    func=AF.Identity, bias=qb_sb[:, dc:dc + 1], scale=1.0)

            def proj_V():
                for ncn in range(NCH):
                    ps = psum.tile([128, 1024], F32, tag="big")
                    for kc in range(KC):
                        nc.tensor.matmul(
                            ps[:, 0:512],
                            lhsT=encT_sb[:, kc, ncn * 128:(ncn + 1) * 128],
                            rhs=vw_sb[:, kc, :],
                            start=(kc == 0), stop=False)
                    nc.tensor.matmul(ps[:, 0:512], lhsT=ones_f, rhs=vb_sb,
                                     start=False, stop=True)
                    nc.scalar.activation(
                        out=V_sb[:, ncn, :, 0:64],
                        in_=ps[:, 0:512].rearrange("p (h d) -> p h d", h=HL),
                        func=AF.Copy)

            def head(h):
                dc, half = divmod(h, 2)
                hb = half * 64
                pv = psum.tile([65, 1024], F32, tag="pv")
                ets = []
                for ncn in range(NCH):
                    sc = psum.tile([128, 1024], F32, tag="big")
                    for mh in range(2):
                        nc.tensor.matmul(
                            sc[:, mh * 512:(mh + 1) * 512],
                            lhsT=kT_sb[hb:hb + 64, dc, ncn * 128:(ncn + 1) * 128],
                            rhs=qT_sb[hb:hb + 64, dc, mh * 512:(mh + 1) * 512],
                            start=True, stop=True)
                    et = etp.tile([128, 1024], BF16, tag="et")
                    ets.append(et)
                    nc.scalar.activation(out=et, in_=sc, func=AF.Exp,
                                         bias=mask_sb[:, ncn:ncn + 1], scale=0.125)
                    for mh in range(2):
                        nc.tensor.matmul(
                            pv[:, mh * 512:(mh + 1) * 512],
                            lhsT=V_sb[:, ncn, h, :],
                            rhs=et[:, mh * 512:(mh + 1) * 512],
                            start=(ncn == 0), stop=(ncn == NCH - 1))
                # softmax denominators -> reciprocal -> broadcast to all partitions
                rzrow = work.tile([1, 1024], F32, tag="rzrow")
                nc.vector.reciprocal(out=rzrow, in_=pv[64:65, :])
                bc = psum.tile([128, 1024], F32, tag="big")
                for mh in range(2):
                    nc.tensor.matmul(bc[:, mh * 512:(mh + 1) * 512], lhsT=ones_f,
                                     rhs=rzrow[:, mh * 512:(mh + 1) * 512],
                                     start=True, stop=True)
                rz = work.tile([128, 1024], BF16, tag="rz")
                nc.scalar.activation(out=rz, in_=bc, func=AF.Copy)
                # normalized attention output slice (pre-c_proj)
                nc.vector.tensor_mul(out=attnT_sb[hb:hb + 64, dc, :],
                                     in0=pv[0:64, :], in1=rz[0:64, :])
                # normalized probabilities -> DRAM (transposed layout)
                for ncn in range(NCH):
                    pt = outs.tile([128, 1024], F32, tag="pt")
                    nc.vector.tensor_mul(out=pt, in0=ets[ncn], in1=rz)
                    nc.sync.dma_start(
                        out=ptout[h, ncn * 128:(ncn + 1) * 128, :], in_=pt)

            # emission order chosen so head 0 can start as soon as its inputs exist
            proj_kT_qT(0)
            proj_V()
            head(0)
            head(1)
            for dc in range(1, DCH):
                proj_kT_qT(dc)
                head(2 * dc)
                head(2 * dc + 1)

            # ---- c_proj partial: outp[m, :] = attnT.T @ pw ----
            for mc in range(MCH):
                ps = psum.tile([128, 1024], F32, tag="big")
                for dc in range(DCH):
                    for nh in range(2):
                        nc.tensor.matmul(
                            ps[:, nh * 512:(nh + 1) * 512],
                            lhsT=attnT_sb[:, dc, mc * 128:(mc + 1) * 128],
                            rhs=pw_sb[:, dc, nh * 512:(nh + 1) * 512],
                            start=(dc == 0), stop=(dc == DCH - 1))
                ost = outs.tile([128, 1024], F32, tag="ost")
                nc.scalar.activation(out=ost, in_=ps, func=AF.Copy)
                nc.sync.dma_start(out=outp[mc * 128:(mc + 1) * 128, :], in_=ost)

    nc.compile()
    return nc


def _get_nc():
    global _NC
    if _NC is None:
        _NC = _build()
    return _NC


def _shard(inputs):
    bf16 = ml_dtypes.bfloat16
    x = np.asarray(inputs["hidden_states"], np.float32)
    enc = np.asarray(inputs["encoder_hidden_states"], np.float32)
    maskf = np.asarray(inputs["attention_mask_kv"], np.float32)
    qw = np.asarray(inputs["q_attn_w"], np.float32)
    qb = np.asarray(inputs["q_attn_b"], np.float32)
    cw = np.asarray(inputs["c_attn_w"], np.float32)
    cb = np.asarray(inputs["c_attn_b"], np.float32)
    pwf = np.asarray(inputs["c_proj_w"], np.float32)

    in_maps = []
    for c in range(8):
        b, g = divmod(c, 2)
        sl = slice(g * DL, (g + 1) * DL)
        vsl = slice(D + g * DL, D + (g + 1) * DL)
        in_maps.append({
            "xT": np.ascontiguousarray(x[b].T).astype(bf16),
            "encT": np.ascontiguousarray(enc[b].T).astype(bf16),
            "qw": np.ascontiguousarray(qw[:, sl]).astype(bf16),
            "kw": np.ascontiguousarray(cw[:, sl]).astype(bf16),
            "vw": np.ascontiguousarray(cw[:, vsl]).astype(bf16),
            "pw": np.ascontiguousarray(pwf[sl, :]).astype(bf16),
            "qb": np.ascontiguousarray(qb[sl].reshape(DCH, 128).T),
            "kb": np.ascontiguousarray(cb[sl].reshape(DCH, 128).T),
            "vb": np.ascontiguousarray(cb[vsl].reshape(1, DL)),
            "mask": np.ascontiguousarray(maskf[b, 0, 0].reshape(NCH, 128).T),
        })
    return in_maps


def kernel(**inputs):
    nc = _get_nc()
    in_maps = _shard(inputs)
    res = run_bass_kernel_spmd(nc, in_maps, core_ids=list(range(8)))

    pb = np.asarray(inputs["c_proj_b"], np.float32)
    attn_w = np.empty((B, H, SQ, SKV), np.float32)
    out = np.zeros((B, SQ, D), np.float32)
    for c in range(8):
        b, g = divmod(c, 2)
        pt = res.results[c]["ptout"]           # [8, SKV, SQ]
        attn_w[b, g * HL:(g + 1) * HL] = pt.transpose(0, 2, 1)
        out[b] += res.results[c]["outp"]
    out += pb
    return out, attn_w


# revision 13
# speedup vs baseline: 1.0970x; 1.0970x over previous
"""Trainium2 Bass kernel for nn_CrossAttention (B=4, SQ=1024, SKV=2048, D=1024, H=16).

Sharding: core c handles batch b = c//2 and head-group g = c%2 (8 heads each).
Per core (all matmuls in bf16 with f32 PSUM accumulation):
  qT  = qw_g^T @ x_b^T          [512d, 1024m]   (d-major so heads slice on partitions)
  kT  = kw_g^T @ enc_b^T        [512d, 2048n]
  V   = enc_b @ vw_g (+vb)      [2048n, 512d]   (+ a column of ones per head)
  per head h (d-slice of 64):
    S^T[n,m] = kT_h^T.T @ qT_h  (2 concurrent K=32 row-tiles, scores transposed)
    E^T      = exp(0.125*S^T + mask[n])          (ACT, bias = per-partition mask)
    [O^T | Z] = [V_h | 1].T @ E^T                (PV matmul; row 64 = softmax denom Z[m])
    rz[128,m] = exp(-broadcast(ln Z))            (1/Z without the slow DVE reciprocal)
    P^T      = E^T * rz         -> attn_weights output (transposed; host views back)
    attnT_h  = O^T * rz[0:64]
  out_partial = attnT.T @ pw_g  [1024m, 1024]   (host sums the 2 partials per batch + bias)

Outputs per core: ptout [8, 2048, 1024] f32 (P^T per head), outp [1024, 1024] f32.
Host: attn_weights[b, g*8+h] = ptout[h].T (numpy view), attn_output = sum of partials + bias.
"""
import sys

if "/opt/trn_rl_repo" not in sys.path:
    sys.path.insert(0, "/opt/trn_rl_repo")

import numpy as np
import ml_dtypes

import concourse.bass as bass
import concourse.tile as tile
from concourse import bacc, mybir
from concourse.bass_utils import run_bass_kernel_spmd

BF16 = mybir.dt.bfloat16
F32 = mybir.dt.float32
AF = mybir.ActivationFunctionType

B, SQ, SKV, D, H = 4, 1024, 2048, 1024, 16
DL = D // 2          # 512 local d per core (8 heads)
KC = D // 128        # 8 contraction chunks
NCH = SKV // 128     # 16 skv chunks
MCH = SQ // 128      # 8 sq chunks
DCH = DL // 128      # 4 local-d chunks
HL = 8               # heads per core
KSPLIT = False       # K=32 row-tiled score matmuls
GP_CHUNKS = 6        # per-head P^T-normalize chunks offloaded to GPSIMD

_NC = None


def _build():
    nc = bacc.Bacc(None, target_bir_lowering=False)

    xT = nc.dram_tensor("xT", [D, SQ], BF16, kind="ExternalInput")
    encT = nc.dram_tensor("encT", [D, SKV], BF16, kind="ExternalInput")
    qw = nc.dram_tensor("qw", [D, DL], BF16, kind="ExternalInput")
    kw = nc.dram_tensor("kw", [D, DL], BF16, kind="ExternalInput")
    vw = nc.dram_tensor("vw", [D, DL], BF16, kind="ExternalInput")
    pw = nc.dram_tensor("pw", [DL, D], BF16, kind="ExternalInput")
    qb = nc.dram_tensor("qb", [128, DCH], F32, kind="ExternalInput")
    kb = nc.dram_tensor("kb", [128, DCH], F32, kind="ExternalInput")
    vb = nc.dram_tensor("vb", [1, DL], F32, kind="ExternalInput")
    mask = nc.dram_tensor("mask", [128, NCH], F32, kind="ExternalInput")

    ptout = nc.dram_tensor("ptout", [HL, SKV, SQ], F32, kind="ExternalOutput")
    outp = nc.dram_tensor("outp", [SQ, D], F32, kind="ExternalOutput")

    with tile.TileContext(nc) as tc:
        import contextlib
        with contextlib.ExitStack() as ctx:
            persist = ctx.enter_context(tc.tile_pool(name="persist", bufs=1))
            inpool = ctx.enter_context(tc.tile_pool(name="inputs", bufs=1))
            work = ctx.enter_context(tc.tile_pool(name="work", bufs=2))
            etp = ctx.enter_context(tc.tile_pool(name="et", bufs=18))
            outs = ctx.enter_context(tc.tile_pool(name="outs", bufs=3))
            psum = ctx.enter_context(tc.tile_pool(name="psum", bufs=2, space="PSUM"))

            # ---- constants / small inputs (needed early by ACT bias reads) ----
            ones_f = persist.tile([1, 128], F32, tag="ones_f")
            nc.vector.memset(ones_f, 1.0)
            qb_sb = persist.tile([128, DCH], F32, tag="qb")
            kb_sb = persist.tile([128, DCH], F32, tag="kb")
            vb_sb = persist.tile([1, DL], F32, tag="vb")
            mask_sb = persist.tile([128, NCH], F32, tag="mask")
            nc.sync.dma_start(out=kb_sb, in_=kb[:, :])
            nc.sync.dma_start(out=qb_sb, in_=qb[:, :])
            nc.sync.dma_start(out=vb_sb, in_=vb[:, :])
            nc.sync.dma_start(out=mask_sb, in_=mask[:, :])

            # ---- big inputs; per-K-chunk loads so matmuls start early ----
            xT_sb = inpool.tile([128, KC, SQ], BF16, tag="xT")
            encT_sb = inpool.tile([128, KC, SKV], BF16, tag="encT")
            qw_sb = inpool.tile([128, KC, DL], BF16, tag="qw")
            kw_sb = inpool.tile([128, KC, DL], BF16, tag="kw")
            vw_sb = inpool.tile([128, KC, DL], BF16, tag="vw")
            pw_sb = persist.tile([128, DCH, D], BF16, tag="pw")
            kw_r = kw[:, :].rearrange("(kc p) d -> p kc d", p=128)
            qw_r = qw[:, :].rearrange("(kc p) d -> p kc d", p=128)
            encT_r = encT[:, :].rearrange("(kc p) n -> p kc n", p=128)
            xT_r = xT[:, :].rearrange("(kc p) m -> p kc m", p=128)
            for kc in range(KC):
                nc.sync.dma_start(out=kw_sb[:, kc, :], in_=kw_r[:, kc, :])
                nc.sync.dma_start(out=encT_sb[:, kc, :], in_=encT_r[:, kc, :])
            for kc in range(KC):
                nc.sync.dma_start(out=qw_sb[:, kc, :], in_=qw_r[:, kc, :])
                nc.sync.dma_start(out=xT_sb[:, kc, :], in_=xT_r[:, kc, :])
            nc.sync.dma_start(out=vw_sb, in_=vw[:, :].rearrange("(kc p) d -> p kc d", p=128))
            nc.sync.dma_start(out=pw_sb, in_=pw[:, :].rearrange("(dc p) n -> p dc n", p=128))

            # ---- persistent intermediates ----
            qT_sb = persist.tile([128, DCH, SQ], BF16, tag="qT")
            kT_sb = persist.tile([128, DCH, SKV], BF16, tag="kT")
            V_sb = persist.tile([128, NCH, HL, 65], BF16, tag="V")
            attnT_sb = persist.tile([128, DCH, SQ], BF16, tag="attnT")
            nc.vector.memset(V_sb, 1.0)  # ones column (idx 64) survives the V copies

            def proj_kT_qT(dc):
                # kT[:, dc, :]: two 1024-wide column groups, 8 K-chunks each
                for nh in range(SKV // 1024):
                    ps = psum.tile([128, 1024], F32, tag="big")
                    for mh in range(2):
                        for kc in range(KC):
                            nc.tensor.matmul(
                                ps[:, mh * 512:(mh + 1) * 512],
                                lhsT=kw_sb[:, kc, dc * 128:(dc + 1) * 128],
                                rhs=encT_sb[:, kc, nh * 1024 + mh * 512:
                                            nh * 1024 + (mh + 1) * 512],
                                start=(kc == 0), stop=(kc == KC - 1))
                    nc.scalar.activation(
                        out=kT_sb[:, dc, nh * 1024:(nh + 1) * 1024], in_=ps,
                        func=AF.Identity, bias=kb_sb[:, dc:dc + 1], scale=1.0)
                ps = psum.tile([128, 1024], F32, tag="big")
                for mh in range(2):
                    for kc in range(KC):
                        nc.tensor.matmul(
                            ps[:, mh * 512:(mh + 1) * 512],
                            lhsT=qw_sb[:, kc, dc * 128:(dc + 1) * 128],
                            rhs=xT_sb[:, kc, mh * 512:(mh + 1) * 512],
                            start=(kc == 0), stop=(kc == KC - 1))
                nc.scalar.activation(
                    out=qT_sb[:, dc, :], in_=ps,
                    func=AF.Identity, bias=qb_sb[:, dc:dc + 1], scale=1.0)

            def proj_V():
                for ncn in range(NCH):
                    ps = psum.tile([128, 1024], F32, tag="big")
                    for kc in range(KC):
                        nc.tensor.matmul(
                            ps[:, 0:512],
                            lhsT=encT_sb[:, kc, ncn * 128:(ncn + 1) * 128],
                            rhs=vw_sb[:, kc, :],
                            start=(kc == 0), stop=False)
                    nc.tensor.matmul(ps[:, 0:512], lhsT=ones_f, rhs=vb_sb,
                                     start=False, stop=True)
                    nc.scalar.activation(
                        out=V_sb[:, ncn, :, 0:64],
                        in_=ps[:, 0:512].rearrange("p (h d) -> p h d", h=HL),
                        func=AF.Copy)

            def head(h):
                dc, half = divmod(h, 2)
                hb = half * 64
                pv = psum.tile([65, 1024], F32, tag="pv")
                ets = []
                for ncn in range(NCH):
                    sc = psum.tile([128, 1024], F32, tag="big")
                    if KSPLIT and hb == 0:
                        # two concurrent K=32 row-tiles (base partitions hb, hb+32)
                        for mh in range(2):
                            for ks in (0, 32):
                                nc.tensor.matmul(
                                    sc[:, mh * 512:(mh + 1) * 512],
                                    lhsT=kT_sb[hb + ks:hb + ks + 32, dc,
                                               ncn * 128:(ncn + 1) * 128],
                                    rhs=qT_sb[hb + ks:hb + ks + 32, dc,
                                              mh * 512:(mh + 1) * 512],
                                    start=(ks == 0), stop=(ks == 32),
                                    tile_position=(hb + ks, 0))
                    else:
                        for mh in range(2):
                            nc.tensor.matmul(
                                sc[:, mh * 512:(mh + 1) * 512],
                                lhsT=kT_sb[hb:hb + 64, dc,
                                           ncn * 128:(ncn + 1) * 128],
                                rhs=qT_sb[hb:hb + 64, dc,
                                          mh * 512:(mh + 1) * 512],
                                start=True, stop=True)
                    et = etp.tile([128, 1024], BF16, tag="et")
                    ets.append(et)
                    nc.scalar.activation(out=et, in_=sc, func=AF.Exp,
                                         bias=mask_sb[:, ncn:ncn + 1], scale=0.125)
                    for mh in range(2):
                        nc.tensor.matmul(pv[:, mh * 512:(mh + 1) * 512],
                                         lhsT=V_sb[:, ncn, h, :],
                                         rhs=et[:, mh * 512:(mh + 1) * 512],
                                         start=(ncn == 0), stop=(ncn == NCH - 1))
                # 1/Z = exp(-ln Z), broadcast to all partitions via K=1 matmul
                lnz = work.tile([1, 1024], F32, tag="lnz")
                nc.scalar.activation(out=lnz, in_=pv[64:65, :], func=AF.Ln)
                bc = psum.tile([128, 1024], F32, tag="big")
                for mh in range(2):
                    nc.tensor.matmul(bc[:, mh * 512:(mh + 1) * 512], lhsT=ones_f,
                                     rhs=lnz[:, mh * 512:(mh + 1) * 512],
                                     start=True, stop=True)
                rz = work.tile([128, 1024], BF16, tag="rz")
                nc.scalar.activation(out=rz, in_=bc, func=AF.Exp, scale=-1.0)
                # normalized attention output slice (pre-c_proj)
                nc.vector.tensor_mul(out=attnT_sb[hb:hb + 64, dc, :],
                                     in0=pv[0:64, :], in1=rz[0:64, :])
                # normalized probabilities -> DRAM (transposed layout)
                for ncn in range(NCH):
                    pt = outs.tile([128, 1024], F32, tag="pt")
                    if ncn % 3 == 2 and GP_CHUNKS:  # offload ~1/3 to idle GPSIMD
                        nc.gpsimd.tensor_mul(out=pt, in0=ets[ncn], in1=rz)
                    else:
                        nc.vector.tensor_mul(out=pt, in0=ets[ncn], in1=rz)
                    nc.sync.dma_start(
                        out=ptout[h, ncn * 128:(ncn + 1) * 128, :], in_=pt)

            # emission order: head 0 can start as soon as kT/qT dc0 exist;
            # V matmuls fill PE gaps (PV waits on V chunk-by-chunk)
            proj_kT_qT(0)
            proj_V()
            head(0)
            head(1)
            for dc in range(1, DCH):
                proj_kT_qT(dc)
                head(2 * dc)
                head(2 * dc + 1)

            # ---- c_proj partial: outp[m, :] = attnT.T @ pw ----
            for mc in range(MCH):
                ps = psum.tile([128, 1024], F32, tag="big")
                for mh in range(2):
                    for dc in range(DCH):
                        nc.tensor.matmul(
                            ps[:, mh * 512:(mh + 1) * 512],
                            lhsT=attnT_sb[:, dc, mc * 128:(mc + 1) * 128],
                            rhs=pw_sb[:, dc, mh * 512:(mh + 1) * 512],
                            start=(dc == 0), stop=(dc == DCH - 1))
                ost = outs.tile([128, 1024], F32, tag="ost")
                nc.scalar.activation(out=ost, in_=ps, func=AF.Copy)
                nc.sync.dma_start(out=outp[mc * 128:(mc + 1) * 128, :], in_=ost)

    nc.compile()
    return nc


def _get_nc():
    global _NC
    if _NC is None:
        _NC = _build()
    return _NC


def _shard(inputs):
    bf16 = ml_dtypes.bfloat16
    x = np.asarray(inputs["hidden_states"], np.float32)
    enc = np.asarray(inputs["encoder_hidden_states"], np.float32)
    maskf = np.asarray(inputs["attention_mask_kv"], np.float32)
    qw = np.asarray(inputs["q_attn_w"], np.float32)
    qb = np.asarray(inputs["q_attn_b"], np.float32)
    cw = np.asarray(inputs["c_attn_w"], np.float32)
    cb = np.asarray(inputs["c_attn_b"], np.float32)
    pwf = np.asarray(inputs["c_proj_w"], np.float32)

    in_maps = []
    for c in range(8):
        b, g = divmod(c, 2)
        sl = slice(g * DL, (g + 1) * DL)
        vsl = slice(D + g * DL, D + (g + 1) * DL)
        in_maps.append({
            "xT": np.ascontiguousarray(x[b].T).astype(bf16),
            "encT": np.ascontiguousarray(enc[b].T).astype(bf16),
            "qw": np.ascontiguousarray(qw[:, sl]).astype(bf16),
            "kw": np.ascontiguousarray(cw[:, sl]).astype(bf16),
            "vw": np.ascontiguousarray(cw[:, vsl]).astype(bf16),
            "pw": np.ascontiguousarray(pwf[sl, :]).astype(bf16),
            "qb": np.ascontiguousarray(qb[sl].reshape(DCH, 128).T),
            "kb": np.ascontiguousarray(cb[sl].reshape(DCH, 128).T),
            "vb": np.ascontiguousarray(cb[vsl].reshape(1, DL)),
            "mask": np.ascontiguousarray(maskf[b, 0, 0].reshape(NCH, 128).T),
        })
    return in_maps


def kernel(**inputs):
    nc = _get_nc()
    in_maps = _shard(inputs)
    res = run_bass_kernel_spmd(nc, in_maps, core_ids=list(range(8)))

    pb = np.asarray(inputs["c_proj_b"], np.float32)
    attn_w = np.empty((B, H, SQ, SKV), np.float32)
    out = np.zeros((B, SQ, D), np.float32)
    for c in range(8):
        b, g = divmod(c, 2)
        pt = res.results[c]["ptout"]           # [8, SKV, SQ]
        attn_w[b, g * HL:(g + 1) * HL] = pt.transpose(0, 2, 1)
        out[b] += res.results[c]["outp"]
    out += pb
    return out, attn_w
